# revision 19
# baseline (speedup 1.0000x reference)
"""Trainium2 Bass kernel for Llama GQA attention (B=1, S=2048, HID=4096,
NH=32, NKV=8, HD=128), tensor-parallel over 8 NeuronCores.

Core c owns Q heads [4c, 4c+4) and KV head c (one GQA group per core).
Each core computes its partial contribution to out = attn_out @ wo (wo is
sharded on its input dim); the host sums the 8 partials.

Layouts (per core):
  xT    [4096, 2048] fp16  : x transposed (hidden on partitions for matmul)
  wq    [4096, 512]  fp16  : columns for this core's 4 heads
  wk/wv [4096, 128]  fp16  : columns for this core's KV head
  wo    [512, 4096]  fp16  : rows for this core's 4 heads
  cosT  [128, 2048]  fp16  : RoPE cos, transposed (head-dim on partitions)
  sinS  [128, 2048]  fp16  : RoPE sin, transposed, sign-folded for rotate_half
  dmask [4, 128, 512] fp16 : 0/1 causal masks for the 4 diagonal block offsets

On-chip: q^T/k^T [hd=128, S] from projections; V^T projected then PE-transposed
to V [s,hd]; scores kept transposed (S^T = [s_k, s_q]) so attn@V needs no
transposes; softmax uses exp(s*scale-2) without per-row max (score range is
bounded), denominators via a ones-vector matmul over the partition axis with
the normalize chain software-pipelined one q-block behind the PE stream.
"""

import sys

sys.path.insert(0, "/opt/trn_rl_repo")

import numpy as np

P = 128
S = 2048
HID = 4096
HPC = 4          # q heads per core
NCORES = 8
KT = HID // P    # 32 k-tiles over hidden dim
KQ = 8           # k-tiles per xT chunk
NKQ = KT // KQ   # 4 chunks
SBLK = S // 512  # 4 blocks of 512 along sequence
ST = S // P      # 16 s-tiles of 128
SCALING = 128 ** -0.5
EXP_BIAS = -2.0  # constant shift inside exp; cancels in normalization


def _build_nc():
    import concourse.bass as bass
    import concourse.mybir as mybir
    import concourse.tile as tile
    from concourse import bacc
    from concourse.masks import make_identity

    f16 = mybir.dt.float16
    f32 = mybir.dt.float32
    Alu = mybir.AluOpType
    Act = mybir.ActivationFunctionType

    nc = bacc.Bacc(
        "TRN2",
        target_bir_lowering=False,
        debug=False,
        enable_asserts=False,
        num_devices=NCORES,
    )

    xT_d = nc.dram_tensor("xT", [HID, S], f16, kind="ExternalInput")
    wq_d = nc.dram_tensor("wq", [HID, HPC * P], f16, kind="ExternalInput")
    wk_d = nc.dram_tensor("wk", [HID, P], f16, kind="ExternalInput")
    wv_d = nc.dram_tensor("wv", [HID, P], f16, kind="ExternalInput")
    wo_d = nc.dram_tensor("wo", [HPC * P, HID], f16, kind="ExternalInput")
    cos_d = nc.dram_tensor("cosT", [P, S], f16, kind="ExternalInput")
    sin_d = nc.dram_tensor("sinS", [P, S], f16, kind="ExternalInput")
    msk_d = nc.dram_tensor("dmask", [4, P, 512], f16, kind="ExternalInput")
    out_d = nc.dram_tensor("out", [S, HID], f32, kind="ExternalOutput")

    with tile.TileContext(nc) as tc:
        with tc.tile_pool(name="const", bufs=1) as constp:
            # fine-grained tiles so cross-phase dependencies resolve per block
            qTs = [[constp.tile([P, 512], f16, tag=f"qT{h}_{sb}", name=f"qT{h}_{sb}")
                    for sb in range(SBLK)] for h in range(HPC)]
            kTs = [constp.tile([P, 512], f16, tag=f"kT{sb}", name=f"kT{sb}")
                   for sb in range(SBLK)]
            Vts = [constp.tile([P, P], f16, tag=f"Vt{st}", name=f"Vt{st}")
                   for st in range(ST)]
            OTs = [constp.tile([P, 512], f16, tag=f"OT{i}", name=f"OT{i}")
                   for i in range(HPC * SBLK)]
            ident = constp.tile([P, P], f16, tag="ident")
            make_identity(nc, ident[:])
            ones16 = constp.tile([P, 1], f16, tag="o16")
            nc.vector.memset(ones16[:], 1.0)
            ones32 = constp.tile([1, P], f32, tag="o32")
            nc.vector.memset(ones32[:], 1.0)
            ebias = constp.tile([P, 1], f32, tag="ebias")
            nc.vector.memset(ebias[:], EXP_BIAS)

            xT_r = xT_d.ap().rearrange("(kt p) s -> p kt s", p=P)
            wq_r = wq_d.ap().rearrange("(kt p) m -> p kt m", p=P)

            # ---- phase 1: Q/K/V projections (+ RoPE on q, k) ----
            with (
                tc.tile_pool(name="p1w", bufs=1) as p1w,
                tc.tile_pool(name="xt", bufs=6) as xtp,
                tc.tile_pool(name="rope", bufs=3) as ropep,
                tc.tile_pool(name="ps1", bufs=1, space="PSUM") as ps1,
            ):
                # split weights per k-chunk so matmuls start after the
                # first 1 MB lands instead of after the full 4 MB
                wq_sb = [p1w.tile([P, KQ, HPC * P], f16, tag=f"wq{kq}", name=f"wq{kq}")
                         for kq in range(NKQ)]
                nc.gpsimd.dma_start(wq_sb[0][:], wq_r[:, 0:KQ, :])
                wk_sb = p1w.tile([P, KT, P], f16, tag="wk")
                nc.gpsimd.dma_start(wk_sb[:], wk_d.ap().rearrange("(kt p) m -> p kt m", p=P))
                wv_sb = p1w.tile([P, KT, P], f16, tag="wv")
                nc.gpsimd.dma_start(wv_sb[:], wv_d.ap().rearrange("(kt p) m -> p kt m", p=P))
                cos_sb = p1w.tile([P, S], f16, tag="cos")
                sin_sb = p1w.tile([P, S], f16, tag="sin")

                def rope(ps, out, scol):
                    """out(f16) = ps*cos + rotate_half(ps)*sin; the sign of
                    rotate_half is folded into sinS on the host."""
                    c = cos_sb[:, scol]
                    sn = sin_sb[:, scol]
                    t1 = ropep.tile([P, 512], f16, tag="t1")
                    nc.vector.tensor_tensor(t1[:], ps[:], c, Alu.mult)
                    t2 = ropep.tile([P, 512], f16, tag="t2")
                    # rotate_half: lanes 0:64 read partitions 64:128, vice versa
                    nc.vector.tensor_tensor(t2[0:64, :], ps[64:128, :], sn[0:64, :], Alu.mult)
                    nc.vector.tensor_tensor(t2[64:128, :], ps[0:64, :], sn[64:128, :], Alu.mult)
                    nc.vector.tensor_tensor(out, t1[:], t2[:], Alu.add)

                for sb in range(SBLK):
                    scol = slice(sb * 512, (sb + 1) * 512)
                    ps_q = [ps1.tile([P, 512], f32, tag=f"psq{h}", name=f"ps_q{h}") for h in range(HPC)]
                    ps_k = ps1.tile([P, 512], f32, tag="psk")
                    ps_vt = ps1.tile([P, 512], f32, tag="psvt")
                    # xT streamed in 1MB chunks; chunk kq is released as soon
                    # as all six accumulation chains have consumed it, so the
                    # next chunk's DMA overlaps compute
                    for kq in range(NKQ):
                        xt = xtp.tile([P, KQ, 512], f16, tag="xt", name=f"xt{kq}")
                        nc.sync.dma_start(xt[:], xT_r[:, kq * KQ:(kq + 1) * KQ, scol])
                        if sb == 0 and kq == 0:
                            # just-in-time loads, after the critical-path DMAs
                            nc.gpsimd.dma_start(cos_sb[:], cos_d.ap())
                            nc.gpsimd.dma_start(sin_sb[:], sin_d.ap())
                        if sb == 0 and kq < NKQ - 1:
                            nc.gpsimd.dma_start(
                                wq_sb[kq + 1][:], wq_r[:, (kq + 1) * KQ:(kq + 2) * KQ, :]
                            )
                        for k in range(KQ):
                            kg = kq * KQ + k
                            st, sp = kg == 0, kg == KT - 1
                            for h in range(HPC):
                                nc.tensor.matmul(
                                    ps_q[h][:],
                                    wq_sb[kq][:, k, h * P:(h + 1) * P],
                                    xt[:, k, :],
                                    start=st, stop=sp,
                                )
                            nc.tensor.matmul(
                                ps_k[:], wk_sb[:, kg, :], xt[:, k, :],
                                start=st, stop=sp,
                            )
                            nc.tensor.matmul(
                                ps_vt[:], wv_sb[:, kg, :], xt[:, k, :],
                                start=st, stop=sp,
                            )
                    for h in range(HPC):
                        rope(ps_q[h], qTs[h][sb][:], scol)
                    rope(ps_k, kTs[sb][:], scol)
                    # V^T [hd, s] -> V [s, hd] via PE transpose
                    vts = ropep.tile([P, 512], f16, tag="vts")
                    nc.vector.tensor_copy(vts[:], ps_vt[:])
                    for j in range(4):
                        ps_tr = ps1.tile([P, P], f16, tag="pstr")
                        nc.tensor.transpose(ps_tr[:], vts[:, j * P:(j + 1) * P], ident[:])
                        nc.vector.tensor_copy(Vts[sb * 4 + j][:], ps_tr[:])

            # ---- phase 2: attention per head, causal (S^T layout) ----
            # Software-pipelined across (head, q-block) "blocks": the QK
            # matmuls of block i (which depend only on qT/kT) interleave in
            # the PE stream with the colsum/V matmuls of block i-1 (whose
            # exp tiles are already materialized), so the PE never stalls
            # on the ScalarE exp chain. exp runs on [128,1024] pairs to
            # amortize the ACTIVATE pipeline-fill overhead.
            with (
                tc.tile_pool(name="p2w", bufs=1) as p2w,
                tc.tile_pool(name="exp", bufs=20) as expp,
                tc.tile_pool(name="norm", bufs=3) as normp,
                tc.tile_pool(name="dramscratch", bufs=3, space="DRAM") as dramp,
                tc.tile_pool(name="ps2", bufs=1, space="PSUM") as ps2,
            ):
                dm_sb = p2w.tile([P, 4, 512], f16, tag="dm")
                nc.sync.dma_start(dm_sb[:], msk_d.ap().rearrange("d p m -> p d m"))

                blocks = [(h, qb) for h in range(HPC) for qb in range(SBLK)]
                NBLK = len(blocks)

                pending = []

                def finish(ent):
                    oti, ps_o, rec = ent
                    # broadcast 1/colsum over partitions via a DRAM bounce
                    # (DMA replicates the row; keeps PSUM banks free)
                    rdram = dramp.tile([1, 512], f32, tag="rdram", name="rdram")
                    nc.sync.dma_start(rdram[:], rec[:])
                    bc = normp.tile([P, 512], f32, tag="bc", name="bc")
                    nc.sync.dma_start(bc[:], rdram[:].to_broadcast((P, 512)))
                    nc.vector.tensor_tensor(OTs[oti][:], ps_o[:], bc[:], Alu.mult)

                def emit_qk(h, qb, g, npair):
                    """QK matmuls + exp for pair g (kb = 2g, 2g+1)."""
                    ps_s = ps2.tile([P, 1024], f32, tag="ps_s", name="ps_s", bufs=2)
                    for t in range(2):
                        kb = 2 * g + t
                        nc.tensor.matmul(
                            ps_s[:, t * 512:(t + 1) * 512],
                            kTs[kb // 4][:, (kb % 4) * P:(kb % 4 + 1) * P],
                            qTs[h][qb][:],
                            start=True,
                            stop=True,
                        )
                    ex = expp.tile([P, 1024], f16, tag="ex", name="ex")
                    nc.scalar.activation(
                        ex[:], ps_s[:], Act.Exp, bias=ebias[:], scale=SCALING
                    )
                    for t in range(2):
                        kb = 2 * g + t
                        d = kb - 4 * qb
                        if d >= 0:  # diagonal block: 0/1 causal mask
                            half = ex[:, t * 512:(t + 1) * 512]
                            nc.vector.tensor_tensor(half, half, dm_sb[:, d, :], Alu.mult)
                    return ex

                prev = None  # (ex_tiles, npair, ps_o, ps_cs, oti)
                for i in range(NBLK + 1):
                    if i < NBLK:
                        h, qb = blocks[i]
                        npair = 2 * qb + 2
                        ex_tiles = []
                    else:
                        npair = 0
                    if prev is not None:
                        pex, pnpair, ps_o, ps_cs, oti = prev
                    else:
                        pnpair = 0
                    for g in range(max(npair, pnpair)):
                        if i < NBLK and g < npair:
                            ex_tiles.append(emit_qk(h, qb, g, npair))
                        if prev is not None and g < pnpair:
                            for t in range(2):
                                kb = 2 * g + t
                                half = pex[g][:, t * 512:(t + 1) * 512]
                                nc.tensor.matmul(
                                    ps_cs[:], ones16[:], half,
                                    start=(kb == 0), stop=(kb == 2 * pnpair - 1),
                                )
                                nc.tensor.matmul(
                                    ps_o[:], Vts[kb][:], half,
                                    start=(kb == 0), stop=(kb == 2 * pnpair - 1),
                                )
                    if prev is not None:
                        rec = normp.tile([1, 512], f32, tag="rec", name="rec")
                        nc.vector.reciprocal_approx_fast(rec[:], ps_cs[:])
                        if pending:
                            finish(pending.pop())
                        pending.append((oti, ps_o, rec))
                    if i < NBLK:
                        ps_o = ps2.tile([P, 512], f32, tag="ps_o", name="ps_o", bufs=2)
                        ps_cs = ps2.tile([1, 512], f32, tag="ps_cs", name="ps_cs", bufs=2)
                        prev = (ex_tiles, npair, ps_o, ps_cs, h * SBLK + qb)
                while pending:
                    finish(pending.pop())

            # ---- phase 3: out_partial = O @ wo (this core's head group) ----
            with (
                tc.tile_pool(name="p3w", bufs=1) as p3w,
                tc.tile_pool(name="stage", bufs=3) as stagep,
                tc.tile_pool(name="ps3", bufs=4, space="PSUM") as ps3,
            ):
                wo_r = wo_d.ap().rearrange("(h p) n -> p h n", p=P)
                wo_sb = []
                for nt in range(HID // 512):
                    w = p3w.tile([P, HPC, 512], f16, tag=f"wo{nt}", name=f"wo{nt}")
                    nc.sync.dma_start(w[:], wo_r[:, :, nt * 512:(nt + 1) * 512])
                    wo_sb.append(w)
                for st in range(ST):
                    srow = slice(st * P, (st + 1) * P)
                    ssl = slice((st % 4) * P, (st % 4 + 1) * P)
                    stage = stagep.tile([P, HID], f32, tag="stage", name="stage")
                    for nt in range(HID // 512):
                        ps_w = ps3.tile([P, 512], f32, tag="ps_w", name="ps_w")
                        for h in range(HPC):
                            nc.tensor.matmul(
                                ps_w[:],
                                OTs[h * SBLK + st // 4][:, ssl],
                                wo_sb[nt][:, h, :],
                                start=(h == 0),
                                stop=(h == HPC - 1),
                            )
                        dst = stage[:, nt * 512:(nt + 1) * 512]
                        if nt % 2 == 0:
                            nc.scalar.copy(dst, ps_w[:])
                        else:
                            nc.vector.tensor_copy(dst, ps_w[:])
                    nc.sync.dma_start(out_d.ap()[srow, :], stage[:])

    nc.compile()
    return nc


_CACHE = {}


def _get_nc():
    if "nc" not in _CACHE:
        _CACHE["nc"] = _build_nc()
    return _CACHE["nc"]


def make_in_maps(hidden_states, cos, sin, wq, wk, wv, wo):
    x = np.asarray(hidden_states)[0]  # [S, HID] fp32
    xT = np.ascontiguousarray(x.T).astype(np.float16)
    cosT = np.ascontiguousarray(np.asarray(cos)[0].T).astype(np.float16)
    sinT = np.ascontiguousarray(np.asarray(sin)[0].T).astype(np.float64)
    sinS = sinT.copy()
    sinS[:64] *= -1.0  # rotate_half sign fold: q'[d<64] -= q[d+64]*sin[d]
    sinS = sinS.astype(np.float16)
    f = np.arange(512)[None, :]
    p = np.arange(P)[:, None]
    dmask = np.stack(
        [(f >= p + d * P).astype(np.float16) for d in range(4)]
    )  # [4, 128, 512]
    wq = np.asarray(wq).astype(np.float16)
    wk = np.asarray(wk).astype(np.float16)
    wv = np.asarray(wv).astype(np.float16)
    wo = np.asarray(wo).astype(np.float16)

    in_maps = []
    for c in range(NCORES):
        in_maps.append(
            {
                "xT": xT,
                "wq": np.ascontiguousarray(wq[:, c * 512:(c + 1) * 512]),
                "wk": np.ascontiguousarray(wk[:, c * P:(c + 1) * P]),
                "wv": np.ascontiguousarray(wv[:, c * P:(c + 1) * P]),
                "wo": np.ascontiguousarray(wo[c * 512:(c + 1) * 512, :]),
                "cosT": cosT,
                "sinS": sinS,
                "dmask": dmask,
            }
        )
    return in_maps


def run(in_maps, trace=False, **kw):
    from concourse.bass_utils import run_bass_kernel_spmd

    nc = _get_nc()
    return run_bass_kernel_spmd(
        nc, in_maps, core_ids=list(range(NCORES)), trace=trace, **kw
    )


def kernel(hidden_states, cos, sin, attn_mask, wq, wk, wv, wo):
    in_maps = make_in_maps(hidden_states, cos, sin, wq, wk, wv, wo)
    res = run(in_maps)
    parts = np.stack([np.asarray(r["out"], dtype=np.float32) for r in res.results])
    out = parts.sum(axis=0, dtype=np.float64).astype(np.float32)
    return out.reshape(1, S, HID)


# revision 24
# speedup vs baseline: 1.0148x; 1.0148x over previous
"""Trainium2 Bass kernel for Llama GQA attention (B=1, S=2048, HID=4096,
NH=32, NKV=8, HD=128), tensor-parallel over 8 NeuronCores.

Core c owns Q heads [4c, 4c+4) and KV head c (one GQA group per core).
Each core computes its partial contribution to out = attn_out @ wo (wo is
sharded on its input dim); the host sums the 8 partials.

Layouts (per core):
  xT    [4096, 2048] fp16  : x transposed (hidden on partitions for matmul)
  wq    [4096, 512]  fp16  : columns for this core's 4 heads
  wk/wv [4096, 128]  fp16  : columns for this core's KV head
  wo    [512, 4096]  fp16  : rows for this core's 4 heads
  cosT  [128, 2048]  fp16  : RoPE cos, transposed (head-dim on partitions)
  sinS  [128, 2048]  fp16  : RoPE sin, transposed, sign-folded for rotate_half
  dmask [4, 128, 512] fp16 : 0/1 causal masks for the 4 diagonal block offsets

On-chip: q^T/k^T [hd=128, S] from projections; V^T projected then PE-transposed
to V [s,hd]; scores kept transposed (S^T = [s_k, s_q]) so attn@V needs no
transposes; softmax uses exp(s*scale-2) without per-row max (score range is
bounded), denominators via a ones-vector matmul over the partition axis with
the normalize chain software-pipelined one q-block behind the PE stream.
"""

import sys

sys.path.insert(0, "/opt/trn_rl_repo")

import numpy as np

P = 128
S = 2048
HID = 4096
HPC = 4          # q heads per core
NCORES = 8
KT = HID // P    # 32 k-tiles over hidden dim
KQ = 8           # k-tiles per xT chunk
NKQ = KT // KQ   # 4 chunks
SBLK = S // 512  # 4 blocks of 512 along sequence
ST = S // P      # 16 s-tiles of 128
SCALING = 128 ** -0.5
EXP_BIAS = -2.0  # constant shift inside exp; cancels in normalization


def _build_nc():
    import concourse.bass as bass
    import concourse.mybir as mybir
    import concourse.tile as tile
    from concourse import bacc
    from concourse.masks import make_identity

    f16 = mybir.dt.float16
    f32 = mybir.dt.float32
    Alu = mybir.AluOpType
    Act = mybir.ActivationFunctionType

    nc = bacc.Bacc(
        "TRN2",
        target_bir_lowering=False,
        debug=False,
        enable_asserts=False,
        num_devices=NCORES,
    )

    xT_d = nc.dram_tensor("xT", [HID, S], f16, kind="ExternalInput")
    wq_d = nc.dram_tensor("wq", [HID, HPC * P], f16, kind="ExternalInput")
    wk_d = nc.dram_tensor("wk", [HID, P], f16, kind="ExternalInput")
    wv_d = nc.dram_tensor("wv", [HID, P], f16, kind="ExternalInput")
    wo_d = nc.dram_tensor("wo", [HPC * P, HID], f16, kind="ExternalInput")
    cos_d = nc.dram_tensor("cosT", [P, S], f16, kind="ExternalInput")
    sin_d = nc.dram_tensor("sinS", [P, S], f16, kind="ExternalInput")
    msk_d = nc.dram_tensor("dmask", [4, P, 512], f16, kind="ExternalInput")
    out_d = nc.dram_tensor("out", [S, HID], f32, kind="ExternalOutput")

    with tile.TileContext(nc) as tc:
        with tc.tile_pool(name="const", bufs=1) as constp:
            # fine-grained tiles so cross-phase dependencies resolve per block
            qTs = [[constp.tile([P, 512], f16, tag=f"qT{h}_{sb}", name=f"qT{h}_{sb}")
                    for sb in range(SBLK)] for h in range(HPC)]
            kTs = [constp.tile([P, 512], f16, tag=f"kT{sb}", name=f"kT{sb}")
                   for sb in range(SBLK)]
            Vts = [constp.tile([P, P], f16, tag=f"Vt{st}", name=f"Vt{st}")
                   for st in range(ST)]
            OTs = [constp.tile([P, 512], f16, tag=f"OT{i}", name=f"OT{i}")
                   for i in range(HPC * SBLK)]
            ident = constp.tile([P, P], f16, tag="ident")
            make_identity(nc, ident[:])
            ones16 = constp.tile([P, 1], f16, tag="o16")
            nc.vector.memset(ones16[:], 1.0)
            ones32 = constp.tile([1, P], f32, tag="o32")
            nc.vector.memset(ones32[:], 1.0)
            ebias = constp.tile([P, 1], f32, tag="ebias")
            nc.vector.memset(ebias[:], EXP_BIAS)

            xT_r = xT_d.ap().rearrange("(kt p) s -> p kt s", p=P)
            wq_r = wq_d.ap().rearrange("(kt p) m -> p kt m", p=P)

            # ---- phase 1: Q/K/V projections (+ RoPE on q, k) ----
            with (
                tc.tile_pool(name="p1w", bufs=1) as p1w,
                tc.tile_pool(name="xt", bufs=6) as xtp,
                tc.tile_pool(name="rope", bufs=3) as ropep,
                tc.tile_pool(name="ps1", bufs=1, space="PSUM") as ps1,
            ):
                # split weights per k-chunk so matmuls start after the
                # first 1 MB lands instead of after the full 4 MB
                wq_sb = [p1w.tile([P, KQ, HPC * P], f16, tag=f"wq{kq}", name=f"wq{kq}")
                         for kq in range(NKQ)]
                nc.gpsimd.dma_start(wq_sb[0][:], wq_r[:, 0:KQ, :])
                wk_sb = p1w.tile([P, KT, P], f16, tag="wk")
                nc.gpsimd.dma_start(wk_sb[:], wk_d.ap().rearrange("(kt p) m -> p kt m", p=P))
                wv_sb = p1w.tile([P, KT, P], f16, tag="wv")
                nc.gpsimd.dma_start(wv_sb[:], wv_d.ap().rearrange("(kt p) m -> p kt m", p=P))
                cos_sb = p1w.tile([P, S], f16, tag="cos")
                sin_sb = p1w.tile([P, S], f16, tag="sin")

                def rope(ps, out, scol):
                    """out(f16) = ps*cos + rotate_half(ps)*sin; the sign of
                    rotate_half is folded into sinS on the host."""
                    c = cos_sb[:, scol]
                    sn = sin_sb[:, scol]
                    t1 = ropep.tile([P, 512], f16, tag="t1")
                    nc.vector.tensor_tensor(t1[:], ps[:], c, Alu.mult)
                    t2 = ropep.tile([P, 512], f16, tag="t2")
                    # rotate_half: lanes 0:64 read partitions 64:128, vice versa
                    nc.vector.tensor_tensor(t2[0:64, :], ps[64:128, :], sn[0:64, :], Alu.mult)
                    nc.vector.tensor_tensor(t2[64:128, :], ps[0:64, :], sn[64:128, :], Alu.mult)
                    nc.vector.tensor_tensor(out, t1[:], t2[:], Alu.add)

                for sb in range(SBLK):
                    scol = slice(sb * 512, (sb + 1) * 512)
                    ps_q = [ps1.tile([P, 512], f32, tag=f"psq{h}", name=f"ps_q{h}") for h in range(HPC)]
                    ps_k = ps1.tile([P, 512], f32, tag="psk")
                    ps_vt = ps1.tile([P, 512], f32, tag="psvt")
                    # xT streamed in 1MB chunks; chunk kq is released as soon
                    # as all six accumulation chains have consumed it, so the
                    # next chunk's DMA overlaps compute
                    for kq in range(NKQ):
                        xt = xtp.tile([P, KQ, 512], f16, tag="xt", name=f"xt{kq}")
                        nc.sync.dma_start(xt[:], xT_r[:, kq * KQ:(kq + 1) * KQ, scol])
                        if sb == 0 and kq == 0:
                            # just-in-time loads, after the critical-path DMAs
                            nc.gpsimd.dma_start(cos_sb[:], cos_d.ap())
                            nc.gpsimd.dma_start(sin_sb[:], sin_d.ap())
                        if sb == 0 and kq < NKQ - 1:
                            nc.gpsimd.dma_start(
                                wq_sb[kq + 1][:], wq_r[:, (kq + 1) * KQ:(kq + 2) * KQ, :]
                            )
                        for k in range(KQ):
                            kg = kq * KQ + k
                            st, sp = kg == 0, kg == KT - 1
                            for h in range(HPC):
                                nc.tensor.matmul(
                                    ps_q[h][:],
                                    wq_sb[kq][:, k, h * P:(h + 1) * P],
                                    xt[:, k, :],
                                    start=st, stop=sp,
                                )
                            nc.tensor.matmul(
                                ps_k[:], wk_sb[:, kg, :], xt[:, k, :],
                                start=st, stop=sp,
                            )
                            nc.tensor.matmul(
                                ps_vt[:], wv_sb[:, kg, :], xt[:, k, :],
                                start=st, stop=sp,
                            )
                    for h in range(HPC):
                        rope(ps_q[h], qTs[h][sb][:], scol)
                    rope(ps_k, kTs[sb][:], scol)
                    # V^T [hd, s] -> V [s, hd] via PE transpose
                    vts = ropep.tile([P, 512], f16, tag="vts")
                    nc.vector.tensor_copy(vts[:], ps_vt[:])
                    for j in range(4):
                        ps_tr = ps1.tile([P, P], f16, tag="pstr")
                        nc.tensor.transpose(ps_tr[:], vts[:, j * P:(j + 1) * P], ident[:])
                        nc.vector.tensor_copy(Vts[sb * 4 + j][:], ps_tr[:])

            # ---- phase 2+3: attention fused with the output projection ----
            # Blocks ordered q-block-major: once all 4 heads of a q-block
            # group are normalized, that group's out = O @ wo chains become
            # eligible and are drip-fed into the PE stream between QK pairs,
            # so the PE fills what would otherwise be exp-latency stalls.
            with (
                tc.tile_pool(name="p2w", bufs=1) as p2w,
                tc.tile_pool(name="exp", bufs=20) as expp,
                tc.tile_pool(name="norm", bufs=4) as normp,
                tc.tile_pool(name="stage", bufs=3) as stagep,
                tc.tile_pool(name="dramscratch", bufs=3, space="DRAM") as dramp,
                tc.tile_pool(name="ps2", bufs=1, space="PSUM") as ps2,
            ):
                dm_sb = p2w.tile([P, 4, 512], f16, tag="dm")
                nc.sync.dma_start(dm_sb[:], msk_d.ap().rearrange("d p m -> p d m"))
                wo_r = wo_d.ap().rearrange("(h p) n -> p h n", p=P)
                wo_sb = []
                for nt in range(HID // 512):
                    w = p2w.tile([P, HPC, 512], f16, tag=f"wo{nt}", name=f"wo{nt}")
                    nc.gpsimd.dma_start(w[:], wo_r[:, :, nt * 512:(nt + 1) * 512])
                    wo_sb.append(w)

                blocks = [(qb, h) for qb in range(SBLK) for h in range(HPC)]
                NBLK = len(blocks)

                pending = []          # normalize chains deferred one block
                wo_queue = []         # (st, nt) output-projection chains
                stage_tiles = {}      # st -> (tile, ndone)
                copy_rr = [0]
                queued_qb = [0]       # q-block groups whose wo chains are queued

                def finish(ent):
                    oti, ps_o, rec = ent
                    # broadcast 1/colsum over partitions via a DRAM bounce
                    rdram = dramp.tile([1, 512], f32, tag="rdram", name="rdram")
                    nc.sync.dma_start(rdram[:], rec[:])
                    bc = normp.tile([P, 512], f32, tag="bc", name="bc")
                    nc.sync.dma_start(bc[:], rdram[:].to_broadcast((P, 512)))
                    nc.vector.tensor_tensor(OTs[oti][:], ps_o[:], bc[:], Alu.mult)

                def emit_wo_chain():
                    if not wo_queue:
                        return
                    st, nt = wo_queue.pop(0)
                    if st not in stage_tiles:
                        stage_tiles[st] = [
                            stagep.tile([P, HID], f32, tag="stage", name="stage"), 0
                        ]
                    stage, _ = stage_tiles[st]
                    ssl = slice((st % 4) * P, (st % 4 + 1) * P)
                    ps_w = ps2.tile([P, 512], f32, tag="ps_w", name="ps_w", bufs=2)
                    for h in range(HPC):
                        nc.tensor.matmul(
                            ps_w[:],
                            OTs[h * SBLK + st // 4][:, ssl],
                            wo_sb[nt][:, h, :],
                            start=(h == 0),
                            stop=(h == HPC - 1),
                        )
                    dst = stage[:, nt * 512:(nt + 1) * 512]
                    copy_rr[0] += 1
                    if copy_rr[0] % 3 == 0:  # 1:2 ACT:DVE balance
                        nc.scalar.copy(dst, ps_w[:])
                    else:
                        nc.vector.tensor_copy(dst, ps_w[:])
                    stage_tiles[st][1] += 1
                    if stage_tiles[st][1] == HID // 512:
                        nc.sync.dma_start(out_d.ap()[st * P:(st + 1) * P, :], stage[:])
                        del stage_tiles[st]

                def emit_qk(h, qb, g):
                    """QK matmuls + exp for pair g (kb = 2g, 2g+1)."""
                    ps_s = ps2.tile([P, 1024], f32, tag="ps_s", name="ps_s", bufs=1)
                    for t in range(2):
                        kb = 2 * g + t
                        nc.tensor.matmul(
                            ps_s[:, t * 512:(t + 1) * 512],
                            kTs[kb // 4][:, (kb % 4) * P:(kb % 4 + 1) * P],
                            qTs[h][qb][:],
                            start=True,
                            stop=True,
                        )
                    ex = expp.tile([P, 1024], f16, tag="ex", name="ex")
                    nc.scalar.activation(
                        ex[:], ps_s[:], Act.Exp, bias=ebias[:], scale=SCALING
                    )
                    for t in range(2):
                        kb = 2 * g + t
                        d = kb - 4 * qb
                        if d >= 0:  # diagonal block: 0/1 causal mask
                            half = ex[:, t * 512:(t + 1) * 512]
                            nc.vector.tensor_tensor(half, half, dm_sb[:, d, :], Alu.mult)
                    return ex

                prev = None  # (ex_tiles, npair, ps_o, ps_cs, oti)
                for i in range(NBLK + 1):
                    # output-projection chains for q-block group qb become
                    # eligible two iterations after the group's last block
                    # (normalize is itself deferred by one block)
                    if i >= 6 and (i - 6) % 4 == 0:
                        qb_ready = (i - 6) // 4
                        queued_qb[0] = qb_ready + 1
                        for st in range(qb_ready * 4, qb_ready * 4 + 4):
                            for nt in range(HID // 512):
                                wo_queue.append((st, nt))
                    if i < NBLK:
                        qb, h = blocks[i]
                        npair = 2 * qb + 2
                        ex_tiles = []
                    else:
                        npair = 0
                    if prev is not None:
                        pex, pnpair, ps_o, ps_cs, oti = prev
                    else:
                        pnpair = 0
                    for g in range(max(npair, pnpair)):
                        if i < NBLK and g < npair:
                            ex_tiles.append(emit_qk(h, qb, g))
                        if prev is not None and g < pnpair:
                            for t in range(2):
                                kb = 2 * g + t
                                half = pex[g][:, t * 512:(t + 1) * 512]
                                nc.tensor.matmul(
                                    ps_cs[:], ones16[:], half,
                                    start=(kb == 0), stop=(kb == 2 * pnpair - 1),
                                )
                                nc.tensor.matmul(
                                    ps_o[:], Vts[kb][:], half,
                                    start=(kb == 0), stop=(kb == 2 * pnpair - 1),
                                )
                        for _ in range(3):
                            emit_wo_chain()
                    if prev is not None:
                        rec = normp.tile([1, 512], f32, tag="rec", name="rec")
                        nc.vector.reciprocal_approx_fast(rec[:], ps_cs[:])
                        if pending:
                            finish(pending.pop())
                        pending.append((oti, ps_o, rec))
                    if i < NBLK:
                        ps_o = ps2.tile([P, 512], f32, tag="ps_o", name="ps_o", bufs=2)
                        ps_cs = ps2.tile([1, 512], f32, tag="ps_cs", name="ps_cs", bufs=1)
                        prev = (ex_tiles, npair, ps_o, ps_cs, h * SBLK + qb)
                while pending:
                    finish(pending.pop())
                for qb in range(queued_qb[0], SBLK):
                    for st in range(qb * 4, qb * 4 + 4):
                        for nt in range(HID // 512):
                            wo_queue.append((st, nt))
                while wo_queue:
                    emit_wo_chain()

    nc.compile()
    return nc


_CACHE = {}


def _get_nc():
    if "nc" not in _CACHE:
        _CACHE["nc"] = _build_nc()
    return _CACHE["nc"]


def make_in_maps(hidden_states, cos, sin, wq, wk, wv, wo):
    x = np.asarray(hidden_states)[0]  # [S, HID] fp32
    xT = np.ascontiguousarray(x.T).astype(np.float16)
    cosT = np.ascontiguousarray(np.asarray(cos)[0].T).astype(np.float16)
    sinT = np.ascontiguousarray(np.asarray(sin)[0].T).astype(np.float64)
    sinS = sinT.copy()
    sinS[:64] *= -1.0  # rotate_half sign fold: q'[d<64] -= q[d+64]*sin[d]
    sinS = sinS.astype(np.float16)
    f = np.arange(512)[None, :]
    p = np.arange(P)[:, None]
    dmask = np.stack(
        [(f >= p + d * P).astype(np.float16) for d in range(4)]
    )  # [4, 128, 512]
    wq = np.asarray(wq).astype(np.float16)
    wk = np.asarray(wk).astype(np.float16)
    wv = np.asarray(wv).astype(np.float16)
    wo = np.asarray(wo).astype(np.float16)

    in_maps = []
    for c in range(NCORES):
        in_maps.append(
            {
                "xT": xT,
                "wq": np.ascontiguousarray(wq[:, c * 512:(c + 1) * 512]),
                "wk": np.ascontiguousarray(wk[:, c * P:(c + 1) * P]),
                "wv": np.ascontiguousarray(wv[:, c * P:(c + 1) * P]),
                "wo": np.ascontiguousarray(wo[c * 512:(c + 1) * 512, :]),
                "cosT": cosT,
                "sinS": sinS,
                "dmask": dmask,
            }
        )
    return in_maps


def run(in_maps, trace=False, **kw):
    from concourse.bass_utils import run_bass_kernel_spmd

    nc = _get_nc()
    return run_bass_kernel_spmd(
        nc, in_maps, core_ids=list(range(NCORES)), trace=trace, **kw
    )


def kernel(hidden_states, cos, sin, attn_mask, wq, wk, wv, wo):
    in_maps = make_in_maps(hidden_states, cos, sin, wq, wk, wv, wo)
    res = run(in_maps)
    parts = np.stack([np.asarray(r["out"], dtype=np.float32) for r in res.results])
    out = parts.sum(axis=0, dtype=np.float64).astype(np.float32)
    return out.reshape(1, S, HID)
